# revision 29
# baseline (speedup 1.0000x reference)
"""Multi-head causal attention block (qkv -> softmax(QK^T/sqrt(d)+mask) V -> proj)
on 8 Trainium2 NeuronCores.

Sharding: 8 cores = 4 batches (data parallel) x 2 head-groups of 8 heads
(tensor parallel: W_qkv column-sharded, W_proj row-sharded). Each core
computes a partial projection output for its (batch, head-group); the host
sums the two partials per batch (the "all-reduce") and adds b_proj.

Core kernel (per core):
  - qT/kT computed in [d, n] layout, v in [n, d] layout (x pre-transposed on
    host so every matmul contracts over the partition dim). x and W_qkv are
    fp16 (input-rounding error only; matmuls accumulate fp32 in PSUM).
  - attention uses transposed scores S^T[k, q] = (kT_tile).T @ qT so that the
    softmax denominator comes for free from a ones-column augmented V
    (out[64] = column sums) and P^T never needs an on-chip transpose.
  - causal structure: fully-masked 128x128 blocks are skipped; on diagonal
    blocks the mask is applied as a post-exp multiply by host-precomputed
    exp(mask) (exp(s+m) = exp(s)*exp(m)), avoiding any PSUM read-modify-write.
  - exp on ScalarE without max subtraction (logits are O(5) here; exact for
    the softmax up to fp rounding).
  - PSUM pools are split by use (scores+proj / PV accumulators / qkv) so the
    Tile scheduler can pull next-stage qkv matmuls into the exp-paced gaps of
    the attention inner loop; qkv chunks for stage ng+1 are emitted after
    proj(ng) at the lowest priority and fill PE idle slots, keeping the PE
    dense (and the HAM clock-gate warm) across stage boundaries.
"""

from collections import deque

import numpy as np

B, N, C = 4, 2048, 1024
H, D = 16, 64
G = 2                  # head groups (cores = B * G = 8)
HPC = H // G           # heads per core
DG = HPC * D           # 512 = per-core qkv width per projection
NT = N // 128          # 16 k/n tiles
QG = N // 512          # 4 q groups
VW = 65                # v_aug width per head (ones col + 64 dims)

_CACHE = {}


def _classify_blocks(attn_mask):
    """Per 128x128 block (j=k-tile, i=q-tile): 0 all-zero, 1 all-masked, 2 mixed."""
    sub = np.empty((NT, NT), dtype=np.int8)
    for j in range(NT):
        for i in range(NT):
            blk = attn_mask[i * 128:(i + 1) * 128, j * 128:(j + 1) * 128]
            if np.all(blk == 0.0):
                sub[j, i] = 0
            elif np.all(blk <= -150.0):
                sub[j, i] = 1
            else:
                sub[j, i] = 2
    return sub


def _build_plan(attn_mask):
    """Plan: for each (qgroup i4, k-tile j) either skip or compute cols
    [lo,hi) (128-units within the 512-wide group) with optional mask add
    (segment id, add_lo, add_hi). Returns plan + concatenated mask segments."""
    sub = _classify_blocks(attn_mask)
    segs = {}
    seg_list = []
    plan = []  # list over i4 of list of (j, lo, hi, mseg or None)
    for i4 in range(QG):
        entries = []
        for j in range(NT):
            states = [sub[j, 4 * i4 + qc] for qc in range(4)]
            keep = [qc for qc in range(4) if states[qc] != 1]
            if not keep:
                continue
            lo, hi = min(keep), max(keep) + 1
            need = [qc for qc in range(lo, hi) if states[qc] != 0]
            mseg = None
            if need:
                alo, ahi = min(need), max(need) + 1
                i0 = (4 * i4 + alo) * 128
                i1 = (4 * i4 + ahi) * 128
                seg = np.exp(np.ascontiguousarray(
                    attn_mask[i0:i1, j * 128:(j + 1) * 128].T).astype(
                        np.float64)).astype(np.float32)
                key = (ahi - alo, seg.tobytes())
                if key not in segs:
                    segs[key] = sum(s.shape[1] // 128 for s in seg_list)
                    seg_list.append(seg)
                mseg = (segs[key], alo, ahi)
            entries.append((j, lo, hi, mseg))
        plan.append(entries)
    if seg_list:
        masks_np = np.concatenate(seg_list, axis=1)
    else:
        masks_np = np.zeros((128, 128), dtype=np.float32)
    return plan, masks_np


def _build_program(plan, mask_width):
    import concourse.mybir as mybir
    import concourse.tile as tile
    from concourse import bacc

    F32 = mybir.dt.float32
    F16 = mybir.dt.float16
    AF = mybir.ActivationFunctionType

    nc = bacc.Bacc("TRN2", target_bir_lowering=False, debug=False, num_devices=8)
    xT = nc.dram_tensor("xT", [C, N], F16, kind="ExternalInput").ap()
    wqkv = nc.dram_tensor("wqkv", [C, 3 * DG], F16, kind="ExternalInput").ap()
    wp = nc.dram_tensor("wp", [DG, C], F16, kind="ExternalInput").ap()
    masks = nc.dram_tensor("masks", [128, mask_width], F16, kind="ExternalInput").ap()
    ones = nc.dram_tensor("ones", [128, 128], F16, kind="ExternalInput").ap()
    out = nc.dram_tensor("out", [N, C], F32, kind="ExternalOutput").ap()

    with tile.TileContext(nc) as tc:
        with (tc.tile_pool(name="pers", bufs=1) as pers,
              tc.tile_pool(name="psS", bufs=2, space="PSUM") as psS,
              tc.tile_pool(name="psV", bufs=2, space="PSUM") as psV,
              tc.tile_pool(name="psQ", bufs=2, space="PSUM") as psQ,
              tc.tile_pool(name="xp", bufs=4) as xp,
              tc.tile_pool(name="wqp", bufs=1) as wqp,
              tc.tile_pool(name="ep", bufs=6) as ep,
              tc.tile_pool(name="aop", bufs=2) as aop,
              tc.tile_pool(name="nrm", bufs=4) as nrm,
              tc.tile_pool(name="wpp", bufs=1) as wpp,
              tc.tile_pool(name="op", bufs=3) as op):
            # fp16 q/k/v: input-rounding error only (~1e-3), fp32 accumulate
            sq = pers.tile([128, 4 * N], F16, tag="sq")
            sk = pers.tile([128, 4 * N], F16, tag="sk")
            sv = pers.tile([128, NT * HPC * VW], F16, tag="sv")
            smask = pers.tile([128, mask_width], F16, tag="smask")
            sones = pers.tile([128, 128], F16, tag="sones")
            swq = wqp.tile([128, 8 * 3 * DG], F16, tag="w")
            swp = wpp.tile([128, 4 * C], F16, tag="wp")

            # ------------- QKV stage: 12 chunks of one [128,512] psum each ---
            def emit_x(ng):
                xh = []
                for half in range(2):  # 2 tiles of 4 c-tiles each
                    xt = xp.tile([128, 4 * 512], F16, tag="x",
                                 name=f"xt{half}")
                    nc.sync.dma_start(
                        xt[:].rearrange("p (kt n) -> p kt n", kt=4),
                        xT.rearrange("(kt p) n -> p kt n", p=128)
                          [:, half * 4:half * 4 + 4, ng * 512:(ng + 1) * 512])
                    xh.append(xt)
                return xh

            def make_chunks(ng, xh):
                """12 chunk closures: q/k for head-pair 0 and all of v
                first so attention on this stage can start after 6."""

                def xslice(kt, a, b):
                    return xh[kt // 4][:, (kt % 4) * 512 + a:(kt % 4) * 512 + b]

                def emit_qk(proj, mt):
                    dst = sq if proj == 0 else sk
                    ps = psQ.tile([128, 512], F32, tag="qk")
                    for kt in range(8):
                        nc.tensor.matmul(
                            ps[:],
                            swq[:, kt * 1536 + proj * DG + mt * 128:
                                kt * 1536 + proj * DG + mt * 128 + 128],
                            xslice(kt, 0, 512),
                            start=(kt == 0), stop=(kt == 7))
                    nc.vector.tensor_copy(
                        dst[:, mt * N + ng * 512:mt * N + ng * 512 + 512],
                        ps[:])

                def emit_v(sub):
                    ps = psQ.tile([128, 512], F32, tag="qk")
                    for kt in range(8):
                        nc.tensor.matmul(
                            ps[:],
                            xslice(kt, sub * 128, sub * 128 + 128),
                            swq[:, kt * 1536 + 1024:kt * 1536 + 1536],
                            start=(kt == 0), stop=(kt == 7))
                    nt_i = ng * 4 + sub
                    nc.vector.tensor_copy(
                        sv[:].rearrange("p (t h c) -> p t h c", h=HPC, c=VW)
                          [:, nt_i:nt_i + 1, :, 0:D],
                        ps[:].rearrange("p (s h c) -> p s h c", s=1, c=D))

                chunks = [lambda p=p, m=m: emit_qk(p, m)
                          for (p, m) in [(0, 0), (1, 0)]]
                chunks += [lambda s=s: emit_v(s) for s in range(4)]
                chunks += [lambda p=p, m=m: emit_qk(p, m)
                           for m in range(1, 4) for p in range(2)]
                return chunks

            def emit_qkv(ng, xh=None):
                if xh is None:
                    xh = emit_x(ng)
                for c in make_chunks(ng, xh):
                    c()

            # sones + stage-0 x first (smallest / needed by everything), then
            # weight DMA sliced by first use: q/k head-pair-0 slices and the
            # v block land first so stage-0 chunks (emitted q0,k0,v0-3 first)
            # start as soon as possible; the bulk follows
            nc.sync.dma_start(sones[:], ones)
            xh0 = emit_x(0)
            swq_v = swq[:].rearrange("p (kt c) -> p kt c", kt=8)
            wqkv_v = wqkv.rearrange("(kt p) c -> p kt c", p=128)
            nc.sync.dma_start(swq_v[:, :, 0:128], wqkv_v[:, :, 0:128])
            nc.sync.dma_start(swq_v[:, :, DG:DG + 128], wqkv_v[:, :, DG:DG + 128])
            nc.sync.dma_start(swq_v[:, :, 2 * DG:3 * DG],
                              wqkv_v[:, :, 2 * DG:3 * DG])
            nc.sync.dma_start(swq_v[:, :, 128:DG], wqkv_v[:, :, 128:DG])
            nc.sync.dma_start(swq_v[:, :, DG + 128:2 * DG],
                              wqkv_v[:, :, DG + 128:2 * DG])
            nc.sync.dma_start(smask[:], masks)
            # ones column (at index 64) for every (n-tile, head)
            nc.vector.tensor_copy(
                sv[:].rearrange("p (t c) -> p t c", c=VW)[:, :, 64:65],
                sones[:])

            # warm-up: dead matmuls on the earliest-landing tiles span the
            # rest of the weight-DMA wait, so the HAM clock gate is already
            # at full rate when the first real matmul issues
            for _ in range(24):
                dmy = psV.tile([VW, 512], F32, tag="pv", name="dmy")
                nc.tensor.matmul(dmy[:, 0:512], sones[:, 0:VW],
                                 xh0[0][:, 0:512], start=True, stop=True)

            emit_qkv(0, xh0)

            nc.sync.dma_start(
                swp[:].rearrange("p (kt c) -> p kt c", kt=4),
                wp.rearrange("(kt p) c -> p kt c", p=128))

            # -------- attention stage (one q-group) --------
            def emit_attention(i4, filler=None):
                attn = aop.tile([128, 4 * 512], F16, tag="attn")
                entries = plan[i4]
                first_j = entries[0][0]
                last_j = entries[-1][0]
                # head pairs: head h0 at PE rows 0-63, h0+1 at rows 64-127;
                # adjacent scores matmuls hit different row groups and run
                # concurrently. PV is emitted one j behind (software
                # pipeline) so it never stalls the PE waiting on exp.
                for h0 in range(0, HPC, 2):
                    hm = h0 // 2
                    ppvs = [psV.tile([VW, 512], F32, tag="pv",
                                     name=f"ppv{hh}")
                            for hh in range(2)]

                    def emit_pv(j, l0, et):
                        for hh in range(2):
                            nc.tensor.matmul(
                                ppvs[hh][:, l0:512],
                                sv[:, (j * HPC + h0 + hh) * VW:
                                   (j * HPC + h0 + hh) * VW + VW],
                                et[:, hh * 512 + l0:hh * 512 + 512],
                                start=(j == first_j), stop=(j == last_j))

                    pending = []
                    for (j, lo, hi, mseg) in entries:
                        l0 = lo * 128
                        pscr = psS.tile([128, 1024], F32, tag="mm2")
                        for hh in range(2):
                            hp = hh * 64
                            nc.tensor.matmul(
                                pscr[:, hh * 512 + l0:hh * 512 + 512],
                                sk[hp:hp + 64,
                                   hm * N + j * 128:hm * N + j * 128 + 128],
                                sq[hp:hp + 64, hm * N + i4 * 512 + l0:
                                   hm * N + i4 * 512 + 512],
                                start=True, stop=True)
                        et = ep.tile([128, 1024], F16, tag="exp")
                        # one exp spanning both heads' banks; cols
                        # [512, 512+l0) are unread garbage
                        nc.scalar.activation(et[:, l0:1024],
                                             pscr[:, l0:1024], AF.Exp)
                        if mseg is not None:
                            soff, alo, ahi = mseg
                            w = (ahi - alo) * 128
                            for hh in range(2):
                                nc.vector.tensor_mul(
                                    et[:, hh * 512 + alo * 128:
                                       hh * 512 + alo * 128 + w],
                                    et[:, hh * 512 + alo * 128:
                                       hh * 512 + alo * 128 + w],
                                    smask[:, soff * 128:soff * 128 + w])
                        pending.append((j, l0, et))
                        # depth-3 software pipeline: by the time PV(j-3) hits
                        # the PE queue its exp is long done, so the PE never
                        # blocks on ScalarE between the scores and PV bursts
                        if len(pending) > 3:
                            emit_pv(*pending.pop(0))
                    while pending:
                        emit_pv(*pending.pop(0))
                    # normalize: rows 0..63 / row 64 (the ones-column sums).
                    # Copy the unnormalized PV to SBUF immediately so the
                    # PSUM accumulator recycles ~2.5us earlier (next head
                    # pair's PV isn't ring-blocked), then scale in place.
                    # Unnormalized values stay < ~7e3, safely inside fp16.
                    for hh in range(2):
                        hp = hh * 64
                        ppv = ppvs[hh]
                        srow = nrm.tile([1, 512], F32, tag="srow")
                        rec = nrm.tile([1, 512], F32, tag="rec")
                        bc = nrm.tile([64, 512], F32, tag="bc")
                        tmp = nrm.tile([64, 512], F16, tag="tmp")
                        nc.vector.tensor_copy(tmp[:], ppv[0:64, :])
                        # custom-DVE reciprocal can't read PSUM; stage the
                        # denominator row through SBUF on ScalarE (idle at
                        # stage ends, where this chain is the critical path)
                        nc.scalar.activation(srow[:], ppv[64:65, :], AF.Copy)
                        nc.vector.reciprocal_approx_fast(rec[:], srow[:])
                        nc.gpsimd.partition_broadcast(bc[:], rec[:])
                        nc.vector.tensor_mul(
                            attn[hp:hp + 64, hm * 512:hm * 512 + 512],
                            tmp[:], bc[:])
                    # next-stage qkv chunks here: their priority lands at the
                    # head-pair boundary, so the PE (and the DVE copies that
                    # recycle the chunk psum ring) never starve behind the
                    # whole attention stage
                    if filler is not None:
                        filler()
                return attn

            def emit_proj(i4, attn):
                # projection for this q-group. Uses the qkv psum pool (not
                # the scores pool) so next-stage scores are never ring-gated
                # behind proj, which itself waits on the normalize chain.
                for sub in range(4):
                    ot = op.tile([128, 1024], F32, tag="out")
                    for fg in range(2):
                        ps = psQ.tile([128, 512], F32, tag="qk")
                        for ct in range(4):
                            nc.tensor.matmul(
                                ps[:],
                                attn[:, ct * 512 + sub * 128:
                                     ct * 512 + sub * 128 + 128],
                                swp[:, ct * C + fg * 512:ct * C + fg * 512 + 512],
                                start=(ct == 0), stop=(ct == 3))
                        nc.vector.tensor_copy(ot[:, fg * 512:fg * 512 + 512],
                                              ps[:])
                        # per-half DMA: the first half ships while the second
                        # half's matmuls run, shortening the kernel tail
                        nc.sync.dma_start(
                            out[i4 * 512 + sub * 128:i4 * 512 + sub * 128 + 128,
                                fg * 512:fg * 512 + 512],
                            ot[:, fg * 512:fg * 512 + 512])

            def emit_dummies(n):
                # dead matmuls into a recycled PV bank: fill work that keeps
                # the PE (and its HAM clock gate) warm where no real work
                # is schedulable yet
                for _ in range(n):
                    dmy = psV.tile([VW, 512], F32, tag="pv", name="dmy")
                    nc.tensor.matmul(dmy[:, 0:512], sv[:, 0:VW],
                                     sq[:, 0:512], start=True, stop=True)

            queue = deque()
            for stage in range(QG):
                if stage + 1 < QG:
                    queue.extend(make_chunks(stage + 1, emit_x(stage + 1)))
                # stage 2 holds back 4 chunks (q/k for head-pairs 2,3 of
                # stage 3) so stage 3's early head-pair boundaries have fill
                # work too; they still land before their consumers
                cap = 2 if stage == 2 else 3
                carry = 4 if stage == 2 else 0

                def filler(cap=cap, last=(stage == QG - 1)):
                    for _ in range(cap):
                        if queue:
                            queue.popleft()()
                        elif last:
                            emit_dummies(3)
                            break

                attn = emit_attention(stage, filler)
                emit_proj(stage, attn)
                while len(queue) > carry:
                    queue.popleft()()
            # keep the PE warm through the final normalize chain so the
            # last projection matmuls run at full rate
            emit_dummies(12)
    nc.compile()
    return nc


def _get_program(attn_mask):
    key = attn_mask.tobytes()
    if key not in _CACHE:
        plan, masks_np = _build_plan(attn_mask)
        nc = _build_program(plan, masks_np.shape[1])
        _CACHE[key] = (nc, masks_np)
    return _CACHE[key]


def _make_in_maps(x, attn_mask, W_qkv, W_proj, masks_np):
    w4 = W_qkv.reshape(C, 3, H, D)
    ones = np.ones((128, 128), dtype=np.float16)
    in_maps = []
    for core in range(8):
        b, g = core // G, core % G
        hs = slice(g * HPC, (g + 1) * HPC)
        wq = (w4[:, 0, hs, :] / np.sqrt(D)).reshape(C, DG)
        wk = w4[:, 1, hs, :].reshape(C, DG)
        wv = w4[:, 2, hs, :].reshape(C, DG)
        in_maps.append({
            "xT": np.ascontiguousarray(x[b].T).astype(np.float16),
            "wqkv": np.ascontiguousarray(
                np.concatenate([wq, wk, wv], axis=1)).astype(np.float16),
            "wp": np.ascontiguousarray(
                W_proj[g * DG:(g + 1) * DG, :]).astype(np.float16),
            "masks": masks_np.astype(np.float16),
            "ones": ones,
        })
    return in_maps


def kernel(x, attn_mask, W_qkv, W_proj, b_proj, **run_kwargs):
    from concourse import bass_utils

    x = np.asarray(x, dtype=np.float32)
    attn_mask = np.asarray(attn_mask, dtype=np.float32)
    W_qkv = np.asarray(W_qkv, dtype=np.float32)
    W_proj = np.asarray(W_proj, dtype=np.float32)
    b_proj = np.asarray(b_proj, dtype=np.float32)

    nc, masks_np = _get_program(attn_mask)
    in_maps = _make_in_maps(x, attn_mask, W_qkv, W_proj, masks_np)

    res = bass_utils.run_bass_kernel_spmd(nc, in_maps, core_ids=list(range(8)),
                                          **run_kwargs)
    outp = np.empty((B, N, C), dtype=np.float32)
    for b in range(B):
        outp[b] = res.results[2 * b]["out"] + res.results[2 * b + 1]["out"] + b_proj
    if run_kwargs:
        kernel.last_result = res
    return outp


# revision 31
# speedup vs baseline: 1.0119x; 1.0119x over previous
"""Multi-head causal attention block (qkv -> softmax(QK^T/sqrt(d)+mask) V -> proj)
on 8 Trainium2 NeuronCores.

Sharding: 8 cores = 4 batches (data parallel) x 2 head-groups of 8 heads
(tensor parallel: W_qkv column-sharded, W_proj row-sharded). Each core
computes a partial projection output for its (batch, head-group); the host
sums the two partials per batch (the "all-reduce") and adds b_proj.

Core kernel (per core):
  - qT/kT computed in [d, n] layout, v in [n, d] layout (x pre-transposed on
    host so every matmul contracts over the partition dim). x and W_qkv are
    fp16 (input-rounding error only; matmuls accumulate fp32 in PSUM).
  - attention uses transposed scores S^T[k, q] = (kT_tile).T @ qT so that the
    softmax denominator comes for free from a ones-column augmented V
    (out[64] = column sums) and P^T never needs an on-chip transpose.
  - causal structure: fully-masked 128x128 blocks are skipped; on diagonal
    blocks the mask is applied as a post-exp multiply by host-precomputed
    exp(mask) (exp(s+m) = exp(s)*exp(m)), avoiding any PSUM read-modify-write.
  - exp on ScalarE without max subtraction (logits are O(5) here; exact for
    the softmax up to fp rounding).
  - PSUM pools are split by use (scores+proj / PV accumulators / qkv) so the
    Tile scheduler can pull next-stage qkv matmuls into the exp-paced gaps of
    the attention inner loop; qkv chunks for stage ng+1 are emitted after
    proj(ng) at the lowest priority and fill PE idle slots, keeping the PE
    dense (and the HAM clock-gate warm) across stage boundaries.
"""

from collections import deque

import numpy as np

B, N, C = 4, 2048, 1024
H, D = 16, 64
G = 2                  # head groups (cores = B * G = 8)
HPC = H // G           # heads per core
DG = HPC * D           # 512 = per-core qkv width per projection
NT = N // 128          # 16 k/n tiles
QG = N // 512          # 4 q groups
VW = 65                # v_aug width per head (ones col + 64 dims)

_CACHE = {}


def _classify_blocks(attn_mask):
    """Per 128x128 block (j=k-tile, i=q-tile): 0 all-zero, 1 all-masked, 2 mixed."""
    sub = np.empty((NT, NT), dtype=np.int8)
    for j in range(NT):
        for i in range(NT):
            blk = attn_mask[i * 128:(i + 1) * 128, j * 128:(j + 1) * 128]
            if np.all(blk == 0.0):
                sub[j, i] = 0
            elif np.all(blk <= -150.0):
                sub[j, i] = 1
            else:
                sub[j, i] = 2
    return sub


def _build_plan(attn_mask):
    """Plan: for each (qgroup i4, k-tile j) either skip or compute cols
    [lo,hi) (128-units within the 512-wide group) with optional mask add
    (segment id, add_lo, add_hi). Returns plan + concatenated mask segments."""
    sub = _classify_blocks(attn_mask)
    segs = {}
    seg_list = []
    plan = []  # list over i4 of list of (j, lo, hi, mseg or None)
    for i4 in range(QG):
        entries = []
        for j in range(NT):
            states = [sub[j, 4 * i4 + qc] for qc in range(4)]
            keep = [qc for qc in range(4) if states[qc] != 1]
            if not keep:
                continue
            lo, hi = min(keep), max(keep) + 1
            need = [qc for qc in range(lo, hi) if states[qc] != 0]
            mseg = None
            if need:
                alo, ahi = min(need), max(need) + 1
                i0 = (4 * i4 + alo) * 128
                i1 = (4 * i4 + ahi) * 128
                seg = np.exp(np.ascontiguousarray(
                    attn_mask[i0:i1, j * 128:(j + 1) * 128].T).astype(
                        np.float64)).astype(np.float32)
                key = (ahi - alo, seg.tobytes())
                if key not in segs:
                    segs[key] = sum(s.shape[1] // 128 for s in seg_list)
                    seg_list.append(seg)
                mseg = (segs[key], alo, ahi)
            entries.append((j, lo, hi, mseg))
        plan.append(entries)
    if seg_list:
        masks_np = np.concatenate(seg_list, axis=1)
    else:
        masks_np = np.zeros((128, 128), dtype=np.float32)
    return plan, masks_np


def _build_program(plan, mask_width):
    import concourse.mybir as mybir
    import concourse.tile as tile
    from concourse import bacc

    F32 = mybir.dt.float32
    F16 = mybir.dt.float16
    AF = mybir.ActivationFunctionType

    nc = bacc.Bacc("TRN2", target_bir_lowering=False, debug=False, num_devices=8)
    xT = nc.dram_tensor("xT", [C, N], F16, kind="ExternalInput").ap()
    wqkv = nc.dram_tensor("wqkv", [C, 3 * DG], F16, kind="ExternalInput").ap()
    wp = nc.dram_tensor("wp", [DG, C], F16, kind="ExternalInput").ap()
    masks = nc.dram_tensor("masks", [128, mask_width], F16, kind="ExternalInput").ap()
    ones = nc.dram_tensor("ones", [128, 128], F16, kind="ExternalInput").ap()
    out = nc.dram_tensor("out", [N, C], F32, kind="ExternalOutput").ap()

    with tile.TileContext(nc) as tc:
        with (tc.tile_pool(name="pers", bufs=1) as pers,
              tc.tile_pool(name="psS", bufs=2, space="PSUM") as psS,
              tc.tile_pool(name="psV", bufs=2, space="PSUM") as psV,
              tc.tile_pool(name="psQ", bufs=2, space="PSUM") as psQ,
              tc.tile_pool(name="xp", bufs=4) as xp,
              tc.tile_pool(name="wqp", bufs=1) as wqp,
              tc.tile_pool(name="ep", bufs=6) as ep,
              tc.tile_pool(name="aop", bufs=2) as aop,
              tc.tile_pool(name="nrm", bufs=4) as nrm,
              tc.tile_pool(name="wpp", bufs=1) as wpp,
              tc.tile_pool(name="op", bufs=3) as op):
            # fp16 q/k/v: input-rounding error only (~1e-3), fp32 accumulate
            sq = pers.tile([128, 4 * N], F16, tag="sq")
            sk = pers.tile([128, 4 * N], F16, tag="sk")
            sv = pers.tile([128, NT * HPC * VW], F16, tag="sv")
            smask = pers.tile([128, mask_width], F16, tag="smask")
            sones = pers.tile([128, 128], F16, tag="sones")
            swq = wqp.tile([128, 8 * 3 * DG], F16, tag="w")
            swp = wpp.tile([128, 4 * C], F16, tag="wp")

            # ------------- QKV stage: 12 chunks of one [128,512] psum each ---
            def emit_x(ng):
                xh = []
                for half in range(2):  # 2 tiles of 4 c-tiles each
                    xt = xp.tile([128, 4 * 512], F16, tag="x",
                                 name=f"xt{half}")
                    nc.sync.dma_start(
                        xt[:].rearrange("p (kt n) -> p kt n", kt=4),
                        xT.rearrange("(kt p) n -> p kt n", p=128)
                          [:, half * 4:half * 4 + 4, ng * 512:(ng + 1) * 512])
                    xh.append(xt)
                return xh

            def make_chunks(ng, xh):
                """12 chunk closures: q/k for head-pair 0 and all of v
                first so attention on this stage can start after 6."""

                def xslice(kt, a, b):
                    return xh[kt // 4][:, (kt % 4) * 512 + a:(kt % 4) * 512 + b]

                def emit_qk(proj, mt):
                    dst = sq if proj == 0 else sk
                    ps = psQ.tile([128, 512], F32, tag="qk")
                    for kt in range(8):
                        nc.tensor.matmul(
                            ps[:],
                            swq[:, kt * 1536 + proj * DG + mt * 128:
                                kt * 1536 + proj * DG + mt * 128 + 128],
                            xslice(kt, 0, 512),
                            start=(kt == 0), stop=(kt == 7))
                    nc.vector.tensor_copy(
                        dst[:, mt * N + ng * 512:mt * N + ng * 512 + 512],
                        ps[:])

                def emit_v(sub):
                    ps = psQ.tile([128, 512], F32, tag="qk")
                    for kt in range(8):
                        nc.tensor.matmul(
                            ps[:],
                            xslice(kt, sub * 128, sub * 128 + 128),
                            swq[:, kt * 1536 + 1024:kt * 1536 + 1536],
                            start=(kt == 0), stop=(kt == 7))
                    nt_i = ng * 4 + sub
                    nc.vector.tensor_copy(
                        sv[:].rearrange("p (t h c) -> p t h c", h=HPC, c=VW)
                          [:, nt_i:nt_i + 1, :, 0:D],
                        ps[:].rearrange("p (s h c) -> p s h c", s=1, c=D))

                chunks = [lambda p=p, m=m: emit_qk(p, m)
                          for (p, m) in [(0, 0), (1, 0)]]
                chunks += [lambda s=s: emit_v(s) for s in range(4)]
                chunks += [lambda p=p, m=m: emit_qk(p, m)
                           for m in range(1, 4) for p in range(2)]
                return chunks

            def emit_qkv(ng, xh=None):
                if xh is None:
                    xh = emit_x(ng)
                for c in make_chunks(ng, xh):
                    c()

            # stage-0 x first (every chunk contracts over all of it), then
            # weight DMA sliced by first use: q/k head-pair-0 slices and the
            # v block land first so stage-0 chunks (emitted q0,k0,v0-3 first)
            # start as soon as possible; the bulk follows
            xh0 = emit_x(0)
            swq_v = swq[:].rearrange("p (kt c) -> p kt c", kt=8)
            wqkv_v = wqkv.rearrange("(kt p) c -> p kt c", p=128)
            nc.sync.dma_start(swq_v[:, :, 0:128], wqkv_v[:, :, 0:128])
            nc.sync.dma_start(swq_v[:, :, DG:DG + 128], wqkv_v[:, :, DG:DG + 128])
            nc.sync.dma_start(swq_v[:, :, 2 * DG:3 * DG],
                              wqkv_v[:, :, 2 * DG:3 * DG])
            nc.sync.dma_start(swq_v[:, :, 128:DG], wqkv_v[:, :, 128:DG])
            nc.sync.dma_start(swq_v[:, :, DG + 128:2 * DG],
                              wqkv_v[:, :, DG + 128:2 * DG])
            nc.sync.dma_start(smask[:], masks)
            nc.sync.dma_start(sones[:], ones)
            # ones column (at index 64) for every (n-tile, head)
            nc.vector.tensor_copy(
                sv[:].rearrange("p (t c) -> p t c", c=VW)[:, :, 64:65],
                sones[:])

            emit_qkv(0, xh0)

            nc.sync.dma_start(
                swp[:].rearrange("p (kt c) -> p kt c", kt=4),
                wp.rearrange("(kt p) c -> p kt c", p=128))

            # -------- attention stage (one q-group) --------
            def emit_attention(i4, filler=None):
                attn = aop.tile([128, 4 * 512], F16, tag="attn")
                entries = plan[i4]
                first_j = entries[0][0]
                last_j = entries[-1][0]
                # head pairs: head h0 at PE rows 0-63, h0+1 at rows 64-127;
                # adjacent scores matmuls hit different row groups and run
                # concurrently. PV is emitted one j behind (software
                # pipeline) so it never stalls the PE waiting on exp.
                for h0 in range(0, HPC, 2):
                    hm = h0 // 2
                    ppvs = [psV.tile([VW, 512], F32, tag="pv",
                                     name=f"ppv{hh}")
                            for hh in range(2)]

                    def emit_pv(j, l0, et):
                        for hh in range(2):
                            nc.tensor.matmul(
                                ppvs[hh][:, l0:512],
                                sv[:, (j * HPC + h0 + hh) * VW:
                                   (j * HPC + h0 + hh) * VW + VW],
                                et[:, hh * 512 + l0:hh * 512 + 512],
                                start=(j == first_j), stop=(j == last_j))

                    pending = []
                    for (j, lo, hi, mseg) in entries:
                        l0 = lo * 128
                        pscr = psS.tile([128, 1024], F32, tag="mm2")
                        for hh in range(2):
                            hp = hh * 64
                            nc.tensor.matmul(
                                pscr[:, hh * 512 + l0:hh * 512 + 512],
                                sk[hp:hp + 64,
                                   hm * N + j * 128:hm * N + j * 128 + 128],
                                sq[hp:hp + 64, hm * N + i4 * 512 + l0:
                                   hm * N + i4 * 512 + 512],
                                start=True, stop=True)
                        et = ep.tile([128, 1024], F16, tag="exp")
                        # one exp spanning both heads' banks; cols
                        # [512, 512+l0) are unread garbage
                        nc.scalar.activation(et[:, l0:1024],
                                             pscr[:, l0:1024], AF.Exp)
                        if mseg is not None:
                            soff, alo, ahi = mseg
                            w = (ahi - alo) * 128
                            for hh in range(2):
                                nc.vector.tensor_mul(
                                    et[:, hh * 512 + alo * 128:
                                       hh * 512 + alo * 128 + w],
                                    et[:, hh * 512 + alo * 128:
                                       hh * 512 + alo * 128 + w],
                                    smask[:, soff * 128:soff * 128 + w])
                        pending.append((j, l0, et))
                        # depth-3 software pipeline: by the time PV(j-3) hits
                        # the PE queue its exp is long done, so the PE never
                        # blocks on ScalarE between the scores and PV bursts
                        if len(pending) > 3:
                            emit_pv(*pending.pop(0))
                    while pending:
                        emit_pv(*pending.pop(0))
                    # normalize: rows 0..63 / row 64 (the ones-column sums).
                    # Copy the unnormalized PV to SBUF immediately so the
                    # PSUM accumulator recycles ~2.5us earlier (next head
                    # pair's PV isn't ring-blocked), then scale in place.
                    # Unnormalized values stay < ~7e3, safely inside fp16.
                    for hh in range(2):
                        hp = hh * 64
                        ppv = ppvs[hh]
                        srow = nrm.tile([1, 512], F32, tag="srow")
                        rec = nrm.tile([1, 512], F32, tag="rec")
                        bc = nrm.tile([64, 512], F32, tag="bc")
                        tmp = nrm.tile([64, 512], F16, tag="tmp")
                        nc.vector.tensor_copy(tmp[:], ppv[0:64, :])
                        # custom-DVE reciprocal can't read PSUM; stage the
                        # denominator row through SBUF on ScalarE (idle at
                        # stage ends, where this chain is the critical path)
                        nc.scalar.activation(srow[:], ppv[64:65, :], AF.Copy)
                        nc.vector.reciprocal_approx_fast(rec[:], srow[:])
                        nc.gpsimd.partition_broadcast(bc[:], rec[:])
                        nc.vector.tensor_mul(
                            attn[hp:hp + 64, hm * 512:hm * 512 + 512],
                            tmp[:], bc[:])
                    # next-stage qkv chunks here: their priority lands at the
                    # head-pair boundary, so the PE (and the DVE copies that
                    # recycle the chunk psum ring) never starve behind the
                    # whole attention stage
                    if filler is not None:
                        filler()
                return attn

            def emit_proj(i4, attn):
                # projection for this q-group. Uses the qkv psum pool (not
                # the scores pool) so next-stage scores are never ring-gated
                # behind proj, which itself waits on the normalize chain.
                for sub in range(4):
                    ot = op.tile([128, 1024], F32, tag="out")
                    for fg in range(2):
                        ps = psQ.tile([128, 512], F32, tag="qk")
                        for ct in range(4):
                            nc.tensor.matmul(
                                ps[:],
                                attn[:, ct * 512 + sub * 128:
                                     ct * 512 + sub * 128 + 128],
                                swp[:, ct * C + fg * 512:ct * C + fg * 512 + 512],
                                start=(ct == 0), stop=(ct == 3))
                        nc.vector.tensor_copy(ot[:, fg * 512:fg * 512 + 512],
                                              ps[:])
                        # per-half DMA: the first half ships while the second
                        # half's matmuls run, shortening the kernel tail
                        nc.sync.dma_start(
                            out[i4 * 512 + sub * 128:i4 * 512 + sub * 128 + 128,
                                fg * 512:fg * 512 + 512],
                            ot[:, fg * 512:fg * 512 + 512])

            def emit_dummies(n):
                # dead matmuls into a recycled PV bank: fill work that keeps
                # the PE (and its HAM clock gate) warm where no real work
                # is schedulable yet
                for _ in range(n):
                    dmy = psV.tile([VW, 512], F32, tag="pv", name="dmy")
                    nc.tensor.matmul(dmy[:, 0:512], sv[:, 0:VW],
                                     sq[:, 0:512], start=True, stop=True)

            queue = deque()
            for stage in range(QG):
                if stage + 1 < QG:
                    queue.extend(make_chunks(stage + 1, emit_x(stage + 1)))
                # stage 2 holds back 4 chunks (q/k for head-pairs 2,3 of
                # stage 3) so stage 3's early head-pair boundaries have fill
                # work too; they still land before their consumers
                cap = 2 if stage == 2 else 3
                carry = 4 if stage == 2 else 0

                def filler(cap=cap, last=(stage == QG - 1)):
                    for _ in range(cap):
                        if queue:
                            queue.popleft()()
                        elif last:
                            emit_dummies(3)
                            break

                attn = emit_attention(stage, filler)
                emit_proj(stage, attn)
                while len(queue) > carry:
                    queue.popleft()()
            # keep the PE warm through the final normalize chain so the
            # last projection matmuls run at full rate
            emit_dummies(12)
    nc.compile()
    return nc


def _get_program(attn_mask):
    key = attn_mask.tobytes()
    if key not in _CACHE:
        plan, masks_np = _build_plan(attn_mask)
        nc = _build_program(plan, masks_np.shape[1])
        _CACHE[key] = (nc, masks_np)
    return _CACHE[key]


def _make_in_maps(x, attn_mask, W_qkv, W_proj, masks_np):
    w4 = W_qkv.reshape(C, 3, H, D)
    ones = np.ones((128, 128), dtype=np.float16)
    in_maps = []
    for core in range(8):
        b, g = core // G, core % G
        hs = slice(g * HPC, (g + 1) * HPC)
        wq = (w4[:, 0, hs, :] / np.sqrt(D)).reshape(C, DG)
        wk = w4[:, 1, hs, :].reshape(C, DG)
        wv = w4[:, 2, hs, :].reshape(C, DG)
        in_maps.append({
            "xT": np.ascontiguousarray(x[b].T).astype(np.float16),
            "wqkv": np.ascontiguousarray(
                np.concatenate([wq, wk, wv], axis=1)).astype(np.float16),
            "wp": np.ascontiguousarray(
                W_proj[g * DG:(g + 1) * DG, :]).astype(np.float16),
            "masks": masks_np.astype(np.float16),
            "ones": ones,
        })
    return in_maps


def kernel(x, attn_mask, W_qkv, W_proj, b_proj, **run_kwargs):
    from concourse import bass_utils

    x = np.asarray(x, dtype=np.float32)
    attn_mask = np.asarray(attn_mask, dtype=np.float32)
    W_qkv = np.asarray(W_qkv, dtype=np.float32)
    W_proj = np.asarray(W_proj, dtype=np.float32)
    b_proj = np.asarray(b_proj, dtype=np.float32)

    nc, masks_np = _get_program(attn_mask)
    in_maps = _make_in_maps(x, attn_mask, W_qkv, W_proj, masks_np)

    res = bass_utils.run_bass_kernel_spmd(nc, in_maps, core_ids=list(range(8)),
                                          **run_kwargs)
    outp = np.empty((B, N, C), dtype=np.float32)
    for b in range(B):
        outp[b] = res.results[2 * b]["out"] + res.results[2 * b + 1]["out"] + b_proj
    if run_kwargs:
        kernel.last_result = res
    return outp
